# revision 7
# baseline (speedup 1.0000x reference)
"""CSSA strip-window attention + LePE depthwise conv, Trainium2 Bass kernel.

Config: B=32, H=W=64, C=64, heads=4, head_dim=16, windows 64x8 (512 tokens),
8 windows/image -> 256 windows total, data-parallel: 32 windows per core.

v3 design (engine-balanced around the PSUM-egress wall):
  - QK^T row-tiled bf16 matmuls; q/k for all 4 heads live on partitions
    0..15 (contraction = head_dim 16), psum tile [128, 1024] = 2 k-chunks
    of ONE head.
  - exp split across BOTH elementwise engines that can read PSUM:
      Act: activation(Exp, scale=SCALE, bias=-CB) -> bf16 pt tile
      DVE: Schraudolph exp: tensor_scalar affine -> int16 bit pattern,
           bitcast to bf16 (p = exp(SCALE*s - CB) * (1 + ~2% sawtooth))
  - AV with p STATIONARY ([128 k, 128 q] slices, FWL weight loads) and a
    tiny moving operand [128 k, 17] = v columns + ones column (sumexp for
    free) -> av psum [128 q, (qb,h,17)] in ONE bank per window,
    accumulated over 4 k-chunks, DMAd straight from PSUM.
  - LePE 3x3 depthwise conv on DVE via scalar_tensor_tensor in bf16
    (4x DVE mode), 2 windows (128 channels) at once.
Host does: window re-layout of q/k/v, and on the way back the softmax
normalization (divide by the sumexp column), LePE add, inverse re-layout.
"""

import sys

for _p in ("/opt/trn_rl_repo", "/root/.axon_site", "/root/.axon_site/_ro/trn_rl_repo"):
    if _p not in sys.path:
        sys.path.append(_p)

import numpy as np

import concourse.bass as bass
import concourse.mybir as mybir
from concourse.tile import TileContext
from concourse.vector_clock import ScopedClock

B, HW, C, HEADS = 32, 64, 64, 4
HS, WS = 64, 8
HD = C // HEADS            # 16
WIN = HS * WS              # 512
NW = (HW // HS) * (HW // WS)   # 8 windows per image
BW = B * NW                # 256 windows
N_CORES = 8
WPC = BW // N_CORES        # 32 windows per core
SCALE = float(HD) ** -0.5
CB = 3.0                   # softmax bias: p = exp(SCALE*s - CB); cancels in num/den
SADJ = 5.76                # Schraudolph bit-offset tuning
NAV = HD + 1               # 17: v dims + ones column (sumexp)
F32 = mybir.dt.float32
BF16 = mybir.dt.bfloat16
I16 = mybir.dt.int16
AF = mybir.ActivationFunctionType
ALU = mybir.AluOpType

# Schraudolph constants: bits_bf16(exp(x)) ~= 128/ln2 * x + 16256 - SADJ
# with x = SCALE*s - CB, folded into an affine of the raw score s.
SCH_A = (128.0 / np.log(2.0)) * SCALE
SCH_B = 16256.0 - (128.0 / np.log(2.0)) * CB - SADJ

# exp-tile engine assignment per window: 8 tiles = (head, chunk-pair).
# 'A' -> Act engine (exact exp), 'D' -> DVE (Schraudolph).
EXP_ASSIGN = "ADAADAAD"


def _install_drain_patch():
    """This container's walrus rejects >1 sync-wait on a Drain ('Too many
    sync wait commands'); split the Tile tail drain's waits across
    single-wait drains."""
    if getattr(TileContext, "_drain_patch_installed", False):
        return

    def _drain_and_barrier(self, tick_clock, wait_clock):
        carrier = self.nc.sync.drain()
        wait_clock.add_sem_waits(
            carrier.ins, ScopedClock({None: tick_clock.global_clock})
        )
        si = carrier.ins.sync_info
        waits = list(si.on_wait) if si is not None else []
        if si is not None and len(waits) > 1:
            si.on_wait = waits[:1]
            for sw in waits[1:]:
                d = self.nc.sync.drain()
                dsi = d.ins.sync_info
                if dsi is None:
                    d.ins.sync_info = mybir.SyncInfo(on_wait=[sw], on_update=[])
                else:
                    dsi.on_wait = [sw]
        self.nc.all_engine_barrier()
        popped = self.nc._tile_sem_poison_stack.pop()
        assert popped is self._sem_poison
        self.nc.clear_and_free_semaphores(list(self.sems.allocated().values()))
        self.nc.all_engine_barrier()

    TileContext._drain_and_barrier = _drain_and_barrier
    TileContext._drain_patch_installed = True


def _split_multi_waits(nc):
    """This container's walrus allows only one sync-wait per instruction.
    Hoist extra waits onto same-engine NoOps inserted just before the
    instruction (sequencer processes them in order, so semantics are
    preserved)."""
    for f in nc.m.functions:
        for bb in f.blocks:
            new_insts = []
            for inst in bb.instructions:
                si = inst.sync_info
                waits = list(si.on_wait) if si is not None else []
                if len(waits) > 1:
                    si.on_wait = waits[-1:]
                    for sw in waits[:-1]:
                        nop = mybir.InstNoOp(
                            name=nc.get_next_instruction_name(), ins=[], outs=[]
                        )
                        nop.engine = inst.engine
                        nop.sync_info = mybir.SyncInfo(on_wait=[sw], on_update=[])
                        nc.register_instruction(nop)
                        new_insts.append(nop)
                new_insts.append(inst)
            bb.instructions[:] = new_insts


def build_nc():
    _install_drain_patch()
    nc = bass.Bass("TRN2", target_bir_lowering=False, debug=False,
                   num_devices=N_CORES)

    # q/k on partitions 0..15 (d), free = (head, {q|k}, 512 tokens).
    qk_d = nc.dram_tensor("qkT", [WPC, HD, HEADS * 2 * WIN], BF16,
                          kind="ExternalInput")
    # AV moving operand: [k-in-chunk 128, (head, chunk, 17)] -- 16 v dims + 1s.
    va_d = nc.dram_tensor("vaug", [WPC, 128, HEADS * 4 * NAV], BF16,
                          kind="ExternalInput")
    # LePE conv input, image layout, 2 windows (128 channels) per tile.
    vi_d = nc.dram_tensor("vimg", [WPC, C, WIN], BF16, kind="ExternalInput")
    wb_d = nc.dram_tensor("wb", [128, 12], F32, kind="ExternalInput")
    # AV output: [q-in-block 128, (qb, head, 17)]; col 16 of each 17 = sumexp.
    oa_d = nc.dram_tensor("out_attn", [WPC, 128, 4 * HEADS * NAV], F32,
                          kind="ExternalOutput")
    ol_d = nc.dram_tensor("out_lepe", [WPC, C, WIN], BF16, kind="ExternalOutput")

    with TileContext(nc) as tc:
        with (
            tc.tile_pool(name="wpool", bufs=1) as wpool,
            tc.tile_pool(name="qpool", bufs=3) as qpool,
            tc.tile_pool(name="vapool", bufs=3) as vapool,
            tc.tile_pool(name="ptpool", bufs=10) as ptpool,
            tc.tile_pool(name="vtpool", bufs=2) as vtpool,
            tc.tile_pool(name="laccpool", bufs=2) as laccpool,
            tc.tile_pool(name="stagepool", bufs=2) as stagepool,
            tc.tile_pool(name="qkpool", bufs=3, space=bass.MemorySpace.PSUM) as qkpool,
            tc.tile_pool(name="avpool", bufs=2, space=bass.MemorySpace.PSUM) as avpool,
        ):
            wb_t = wpool.tile([128, 12], F32)
            nc.sync.dma_start(wb_t[:], wb_d.ap())

            state = {}

            def qk_phase(w, spare):
                """QK^T + exp for window w; 8 psum tiles of [128, 1024]
                (= 2 k-chunks of one head). `spare` thunks (prev window's AV
                matmuls) are interleaved to fill PE WAR gaps on psum reuse."""
                qkt = qpool.tile([HD, HEADS * 2 * WIN], BF16, tag="qkt")
                nc.sync.dma_start(qkt[:], qk_d.ap()[w])
                va = vapool.tile([128, HEADS * 4 * NAV], BF16, tag="va")
                nc.sync.dma_start(va[:], va_d.ap()[w])
                pts = {}
                si = 0
                slot = 0
                for h in range(HEADS):
                    qoff = h * 2 * WIN
                    koff = qoff + WIN
                    for cp in range(2):
                        qk = qkpool.tile([128, 1024], F32, tag="qk")
                        for t in range(2):
                            c = 2 * cp + t
                            nc.tensor.matmul(
                                qk[:, 512 * t: 512 * (t + 1)],
                                lhsT=qkt[:, koff + 128 * c: koff + 128 * (c + 1)],
                                rhs=qkt[:, qoff: qoff + WIN],
                                start=True, stop=True,
                            )
                        if EXP_ASSIGN[slot] == "A":
                            pt = ptpool.tile([128, 1024], BF16, tag="ptA")
                            nc.scalar.activation(pt[:], qk[:], AF.Exp,
                                                 scale=SCALE,
                                                 bias=wb_t[:, 10:11])
                            pts[(h, cp)] = (pt, False)
                        else:
                            pt = ptpool.tile([128, 1024], I16, tag="ptD")
                            nc.vector.tensor_scalar(
                                out=pt[:], in0=qk[:],
                                scalar1=float(SCH_A), scalar2=float(SCH_B),
                                op0=ALU.mult, op1=ALU.add,
                            )
                            pts[(h, cp)] = (pt, True)
                        slot += 1
                        n = (len(spare) * slot + 7) // 8
                        while si < n:
                            spare[si]()
                            si += 1
                state[w] = (pts, va)
                while si < len(spare):
                    spare[si]()
                    si += 1

            def av_work(w):
                """Window w's 64 AV matmuls (p stationary [128,128], va moving
                [128,17]) as thunks; the last one adds the PSUM->HBM DMA."""
                if w < 0:
                    return []
                pts, va = state.pop(w)
                av = avpool.tile([128, 512], F32, tag="av")

                thunks = []
                for h in range(HEADS):
                    for qb in range(4):
                        for c in range(4):
                            cp, t = c // 2, c % 2
                            last = (h, qb, c) == (HEADS - 1, 3, 3)

                            def thunk(h=h, qb=qb, c=c, cp=cp, t=t, last=last):
                                pt, is_dve = pts[(h, cp)]
                                lhsT = pt[:, 512 * t + 128 * qb:
                                             512 * t + 128 * (qb + 1)]
                                if is_dve:
                                    lhsT = lhsT.bitcast(BF16)
                                nc.tensor.matmul(
                                    av[:, (qb * HEADS + h) * NAV:
                                          (qb * HEADS + h + 1) * NAV],
                                    lhsT=lhsT,
                                    rhs=va[:, (h * 4 + c) * NAV:
                                              (h * 4 + c + 1) * NAV],
                                    start=(c == 0), stop=(c == 3),
                                )
                                if last:
                                    stage = stagepool.tile(
                                        [128, 4 * HEADS * NAV], F32, tag="st")
                                    nc.vector.tensor_copy(
                                        stage[:], av[:, : 4 * HEADS * NAV])
                                    nc.gpsimd.dma_start(oa_d.ap()[w], stage[:])
                            thunks.append(thunk)
                return thunks

            def conv_work(p):
                """LePE conv on GPSIMD (bf16 input, f32 accumulation)."""
                vt = vtpool.tile([128, WIN], BF16, tag="vt")
                nc.sync.dma_start(
                    vt[:],
                    vi_d.ap().rearrange("w c f -> (w c) f")[
                        (2 * p) * C: (2 * p + 2) * C, :],
                )
                vt3 = vt[:].rearrange("c (h ww) -> c h ww", ww=WS)
                acc = laccpool.tile([128, WIN], BF16, tag="lacc")
                acc3 = acc[:].rearrange("c (h ww) -> c h ww", ww=WS)
                nc.vector.tensor_scalar(
                    out=acc3, in0=vt3,
                    scalar1=wb_t[:, 4:5], scalar2=wb_t[:, 9:10],
                    op0=ALU.mult, op1=ALU.add,
                )
                for tap in range(9):
                    if tap == 4:
                        continue
                    dh, dw = tap // 3 - 1, tap % 3 - 1
                    h0, h1 = max(0, -dh), HS - max(0, dh)
                    w0, w1 = max(0, -dw), WS - max(0, dw)
                    nc.vector.scalar_tensor_tensor(
                        out=acc3[:, h0:h1, w0:w1],
                        in0=vt3[:, h0 + dh: h1 + dh, w0 + dw: w1 + dw],
                        scalar=wb_t[:, tap: tap + 1],
                        in1=acc3[:, h0:h1, w0:w1],
                        op0=ALU.mult, op1=ALU.add,
                    )
                nc.gpsimd.dma_start(
                    ol_d.ap().rearrange("w c f -> (w c) f")[
                        (2 * p) * C: (2 * p + 2) * C, :],
                    acc[:],
                )
                return []

            for w in range(WPC):
                spare = av_work(w - 1)
                if w % 2 == 0:
                    conv_work(w // 2)
                qk_phase(w, spare)
            for thunk in av_work(WPC - 1):
                thunk()

    _split_multi_waits(nc)
    return nc


def host_prep(qkv, w_conv, b_conv):
    """Full inputs -> per-core input maps."""
    import ml_dtypes
    qkv = np.ascontiguousarray(qkv, dtype=np.float32)
    # [3, B, n, c] -> windows: n = h*64 + ww*8 + ws
    x = qkv.reshape(3, B, HS, HW // WS, WS, C)
    x = x.transpose(0, 1, 3, 2, 4, 5)          # [3, b, ww, h, ws, c]
    x = x.reshape(3, BW, WIN, HEADS, HD)       # [3, win, t, head, d]

    # [win, d(16), head, {q|k}, t] -> [win, 16, head*2*512]
    qkT = np.empty((BW, HD, HEADS, 2, WIN), dtype=np.float32)
    qkT[:, :, :, 0, :] = x[0].transpose(0, 3, 2, 1)
    qkT[:, :, :, 1, :] = x[1].transpose(0, 3, 2, 1)
    qkT = qkT.reshape(BW, HD, HEADS * 2 * WIN)

    # AV moving operand: [win, k-in-chunk(128), head, chunk, 17]
    va = np.zeros((BW, 4, 128, HEADS, NAV), dtype=np.float32)
    va[:, :, :, :, :HD] = x[2].reshape(BW, 4, 128, HEADS, HD)
    va[:, :, :, :, HD] = 1.0
    va = va.transpose(0, 2, 3, 1, 4)           # [win, 128, head, chunk, 17]
    va = np.ascontiguousarray(va.reshape(BW, 128, HEADS * 4 * NAV))

    # conv image layout: [win, c, t] with t = h*8 + ws
    vi = np.ascontiguousarray(x[2].reshape(BW, WIN, C).transpose(0, 2, 1))

    wb = np.zeros((128, 12), dtype=np.float32)
    wb[:, 10] = -CB
    taps = np.asarray(w_conv, dtype=np.float32).reshape(C, 9)  # [c, dh*3+dw]
    wb[:64, :9] = taps
    wb[64:, :9] = taps
    wb[:64, 9] = np.asarray(b_conv, dtype=np.float32)
    wb[64:, 9] = np.asarray(b_conv, dtype=np.float32)

    qkT = qkT.astype(ml_dtypes.bfloat16)
    va = va.astype(ml_dtypes.bfloat16)
    vi = vi.astype(ml_dtypes.bfloat16)

    in_maps = []
    for core in range(N_CORES):
        s = slice(core * WPC, (core + 1) * WPC)
        in_maps.append({
            "qkT": qkT[s], "vaug": va[s], "vimg": vi[s], "wb": wb,
        })
    return in_maps


def host_post(results):
    """Per-core outputs -> full [B, HW*HW, C] output."""
    oa = np.concatenate([r["out_attn"] for r in results], axis=0)
    # [BW, 128, (qb, head, 17)] -> [BW, q(512), head, 17]
    oa = oa.reshape(BW, 128, 4, HEADS, NAV).transpose(0, 2, 1, 3, 4)
    oa = oa.reshape(BW, WIN, HEADS, NAV)
    ol = np.concatenate(
        [np.asarray(r["out_lepe"], dtype=np.float32) for r in results], axis=0)
    num = oa[:, :, :, :HD]
    den = oa[:, :, :, HD:]
    att = num / den                                      # [win, t, head, d]
    lepe = ol.reshape(BW, HEADS, HD, WIN).transpose(0, 3, 1, 2)
    y = att + lepe                                       # [win, t, head, d]
    y = y.reshape(B, NW, HS, WS, C)                      # [b, ww, h, ws, c]
    y = y.transpose(0, 2, 1, 3, 4)                       # [b, h, ww, ws, c]
    return np.ascontiguousarray(y.reshape(B, HW * HW, C))


_NC_CACHE = None


def kernel(qkv, w_conv, b_conv):
    global _NC_CACHE
    from concourse.bass_utils import run_bass_kernel_spmd

    if _NC_CACHE is None:
        _NC_CACHE = build_nc()
    in_maps = host_prep(qkv, w_conv, b_conv)
    res = run_bass_kernel_spmd(
        _NC_CACHE, in_maps, core_ids=list(range(N_CORES)), trace=False
    )
    return host_post(res.results)


if __name__ == "__main__":
    rng = np.random.default_rng(0)
    qkv = rng.standard_normal((3, B, HW * HW, C), dtype=np.float32)
    w_conv = (rng.standard_normal((C, 1, 3, 3)) * 0.1).astype(np.float32)
    b_conv = (rng.standard_normal((C,)) * 0.1).astype(np.float32)
    out = kernel(qkv, w_conv, b_conv)
    print("out", out.shape, out.dtype)
